# revision 1
# baseline (speedup 1.0000x reference)
"""CPMLoss (cross-modal center / margin-ranking loss) on 8 Trainium2 NeuronCores.

Strategy (feature-dim sharding):
  - The [8192, 4096] input is sharded along the feature dim D: core c gets the
    contiguous column slice [:, c*512:(c+1)*512] (16.8 MB per core, streamed
    once; the kernel is HBM-read bound).
  - Each core computes, over its D-slice:
      * per-modality/identity centers  c[m]  [128 ids, 512]:
        rows are loaded 4-consecutive-per-partition (fully contiguous DMA,
        8KB descriptors), summed 4:1 inside each partition with exact fp32
        DVE adds, then reduced 4-partitions:1-group with a small fp32 PE
        indicator matmul.  (A single fp32 PE matmul over all 16 rows would
        run at 4 cycles/row and become the kernel bottleneck.)
      * partial Gram matrices  G_m = c_m @ c_m^T  (PE fp32, via PE-transposed
        centers)
      * partial squared norms  s_m[i] = sum_d c_m[i,d]^2  (ACT Square+accum)
      * partial cross-modality diagonal products dp_ab[i] = sum_d c_a*c_b
    All of these are sums over D; two small AllReduces complete the
    reduction: modalities 0+1 fire mid-sweep (fully hidden under the
    remaining DMA), modalities 2+3 + diag products at the end, so only one
    ~10us collective latency is exposed.
  - P x P distance post-processing for modalities 0/1 also overlaps the
    sweep; only modality 2/3 post + the final scalar combine trail the last
    AllReduce.  Every core computes the same scalar; host takes core 0's.
"""

import numpy as np

for _p in ("/opt/trn_rl_repo",):
    import sys

    if _p not in sys.path:
        sys.path.append(_p)

ROWS = 8192          # 4 modalities x 128 identities x 16 samples
D_FULL = 4096
N_CORES = 8
D_LOC = D_FULL // N_CORES   # 512
P_ID = 128           # identities per modality
MODS = 4
K_SAMP = 16
MARGIN = 0.2
# (a, b) modality pairs whose diagonal distances feed the loss:
# j=0: d(c2,c3)=ap123, j=1: d(c1,c3)=an123, j=2: d(c1,c4)=ap124, j=3: d(c2,c4)=an124
PAIRS = ((1, 2), (0, 2), (0, 3), (1, 3))

_PROGRAM = None


def _build_program(bench_reps=0, xp_bufs=4, wq_bufs=4, psc_bufs=2,
                   staggered=False, parts="full", stage=None, gp_add=False, half_dma=False, no_ar=False):
    import contextlib

    import concourse.bass as bass
    import concourse.mybir as mybir
    from concourse import bacc, tile

    f32 = mybir.dt.float32
    Alu = mybir.AluOpType
    Act = mybir.ActivationFunctionType

    nc = bacc.Bacc(
        "TRN2", target_bir_lowering=False, debug=False, num_devices=N_CORES
    )

    x = nc.dram_tensor("x0", [ROWS, D_LOC], f32, kind="ExternalInput")
    loss = nc.dram_tensor("loss", [1, 1], f32, kind="ExternalOutput")

    # --- constants baked into the NEFF ---
    # eq[p, p//4] = 1/16: sums quads of partitions into the 32 slab-groups
    # (each partition already holds the sum of 4 consecutive rows).
    eq_np = np.zeros((128, 32), np.float32)
    for p in range(128):
        eq_np[p, p // 4] = 1.0 / K_SAMP
    id_np = np.eye(128, dtype=np.float32)
    dg_np = np.zeros((128, 256), np.float32)
    np.fill_diagonal(dg_np[:, 0:128], 1.0e30)
    np.fill_diagonal(dg_np[:, 128:256], 1.0e30)
    on_np = np.ones((128, 1), np.float32)
    wv_np = (
        np.array([[0.5, 0.25, 0.25, 0.5, 0.25, 0.25]], np.float32) / 128.0
    )
    eq_d = nc.inline_tensor(eq_np, "eq_const")
    id_d = nc.inline_tensor(id_np, "id_const")
    dg_d = nc.inline_tensor(dg_np, "dg_const")
    on_d = nc.inline_tensor(on_np, "on_const")
    wv_d = nc.inline_tensor(wv_np, "wv_const")

    # stats tile layouts (one reduction buffer per AllReduce chunk):
    #   A (modalities 0,1): [0:128) H0, [128:256) H1, 256 s0, 257 s1
    #   B (modality 2):     [0:128) H2, 128 s2, 129 dp0, 130 dp1
    #   C (modality 3):     [0:128) H3, 128 s3, 129 dp2, 130 dp3
    W_A, W_B = 258, 131

    with tile.TileContext(nc) as tc:
        with (
            tc.tile_pool(name="constp", bufs=1) as constp,
            tc.tile_pool(name="cenp", bufs=1) as cenp,
            tc.tile_pool(name="xp", bufs=xp_bufs) as xp,
            tc.tile_pool(name="wq", bufs=wq_bufs) as wq,
            tc.tile_pool(name="wp", bufs=2) as wp,
            tc.tile_pool(name="psc", bufs=psc_bufs, space="PSUM") as psc,
            tc.tile_pool(name="pst", bufs=2, space="PSUM") as pst,
            tc.tile_pool(name="psg", bufs=2, space="PSUM") as psg,
            tc.tile_pool(name="pss", bufs=1, space="PSUM") as pss,
            tc.tile_pool(name="dramp", bufs=1, space="DRAM") as dramp,
        ):
            eq_sb = constp.tile([128, 32], f32, tag="eq")
            id_sb = constp.tile([128, 128], f32, tag="id")
            dg_sb = constp.tile([128, 256], f32, tag="dg")
            on_sb = constp.tile([128, 1], f32, tag="on")
            wv_sb = constp.tile([1, 6], f32, tag="wv")
            nc.gpsimd.dma_start(eq_sb[:], eq_d[:])
            nc.gpsimd.dma_start(id_sb[:], id_d[:])
            nc.gpsimd.dma_start(dg_sb[:], dg_d[:])
            nc.gpsimd.dma_start(on_sb[:], on_d[:])
            nc.gpsimd.dma_start(wv_sb[:], wv_d[:])

            cen = [
                cenp.tile([128, D_LOC], f32, tag=f"cen{m}", name=f"cen{m}")
                for m in range(MODS)
            ]
            cT = [
                cenp.tile([128, D_LOC], f32, tag=f"ct{m}", name=f"ct{m}")
                for m in range(MODS)
            ]
            stats_a = cenp.tile([128, W_A], f32, tag="stats_a", name="stats_a")
            stats_b = cenp.tile([128, W_B], f32, tag="stats_b", name="stats_b")
            stats_c = cenp.tile([128, W_B], f32, tag="stats_c", name="stats_c")
            rst_a = cenp.tile([128, W_A], f32, tag="rst_a", name="rst_a")
            rst_b = cenp.tile([128, W_B], f32, tag="rst_b", name="rst_b")
            rst_c = cenp.tile([128, W_B], f32, tag="rst_c", name="rst_c")
            rcat = cenp.tile([128, N_CORES, W_B], f32, tag="rcat", name="rcat")
            anm = cenp.tile([128, 4], f32, tag="anm", name="anm")
            pd = cenp.tile([128, 4], f32, tag="pd", name="pd")

            do_ar = bench_reps == 0 and parts == "full" and not no_ar
            if not do_ar:
                # bench mode: collectives cannot live inside a For_i loop;
                # post-process the local partials instead (same op costs).
                red_a, red_b, red_c = stats_a, stats_b, stats_c
            else:
                red_a, red_b, red_c = rst_a, rst_b, rst_c

            def stats_tile(m):
                return (stats_a, stats_a, stats_b, stats_c)[m]

            def scol(m):
                return 256 + m if m < 2 else 128

            def g_ap(m):
                t = (red_a, red_a, red_b, red_c)[m]
                off = (m % 2) * 128 if m < 2 else 0
                return t[:, off : off + 128]

            def s_ap(m):
                t = (red_a, red_a, red_b, red_c)[m]
                c = scol(m)
                return t[:, c : c + 1]

            def dp_ap(j):
                t = red_b if j < 2 else red_c
                return t[:, 129 + (j % 2) : 130 + (j % 2)]

            def dp_store(j):
                return (stats_b if j < 2 else stats_c), 129 + (j % 2)

            def _all_gather_sum(sb_tile, dst, width, name):
                ag_in = dramp.tile([128, width], f32, tag=f"gi_{name}",
                                   name=f"gi_{name}")
                ag_out = dramp.tile([128 * N_CORES, width], f32,
                                    tag=f"go_{name}", name=f"go_{name}")
                nc.gpsimd.dma_start(ag_in[:], sb_tile[:])
                nc.gpsimd.collective_compute(
                    "AllGather",
                    Alu.bypass,
                    replica_groups=[list(range(N_CORES))],
                    ins=[ag_in.opt()],
                    outs=[ag_out.opt()],
                )
                # ranks land on the partition axis [r*128+p, c]; bring them
                # side-by-side in the free dim and sum on DVE
                nc.gpsimd.dma_start(
                    rcat[:], ag_out[:].rearrange("(r p) c -> p r c", r=N_CORES)
                )
                nc.vector.tensor_add(dst[:], rcat[:, 0, :], rcat[:, 1, :])
                for rr in range(2, N_CORES):
                    nc.vector.tensor_add(dst[:], dst[:], rcat[:, rr, :])

            def _all_reduce(sb_tile, dst, width, name):
                ar_in = dramp.tile([128, width], f32, tag=f"ai_{name}",
                                   name=f"ai_{name}")
                ar_out = dramp.tile([128, width], f32, tag=f"ao_{name}",
                                    name=f"ao_{name}")
                nc.gpsimd.dma_start(ar_in[:], sb_tile[:])
                nc.gpsimd.collective_compute(
                    "AllReduce",
                    Alu.add,
                    replica_groups=[list(range(N_CORES))],
                    ins=[ar_in.opt()],
                    outs=[ar_out.opt()],
                )
                nc.gpsimd.dma_start(dst[:], ar_out[:])

            def _post_one(m):
                # an_mm[m]; g_ap() holds H = s_i - G after AR; d2 = H + H^T.
                # min and sqrt commute (both monotone), so take the off-diag
                # row-min on d2 and sqrt only the [128,1] result.
                d = wp.tile([128, 128], f32, tag="d", name="d")
                pt = pst.tile([128, 128], f32, tag="pt", name="pt")
                nc.tensor.transpose(pt[:], g_ap(m), id_sb[:])
                nc.vector.tensor_tensor(d[:], g_ap(m), pt[:], op=Alu.add)
                nc.vector.tensor_scalar(d[:], d[:], 1.0e-12, None, Alu.max)
                nc.vector.tensor_tensor(d[:], d[:], dg_sb[:, 0:128], op=Alu.add)
                nc.vector.tensor_reduce(
                    anm[:, m : m + 1], d[:], axis=mybir.AxisListType.X, op=Alu.min
                )
                nc.scalar.activation(
                    anm[:, m : m + 1], anm[:, m : m + 1], Act.Sqrt
                )

            def _pair_dp(j, a, b):
                pr = wp.tile([128, D_LOC], f32, tag="pr", name="pr")
                nc.vector.tensor_tensor(
                    pr[:], cen[a][:], cen[b][:], op=Alu.mult
                )
                st, col = dp_store(j)
                nc.vector.tensor_reduce(
                    st[:, col : col + 1],
                    pr[:],
                    axis=mybir.AxisListType.X,
                    op=Alu.add,
                )

            # slab i (512 rows), partition p holds rows i*512 + 4p .. 4p+4
            # (fully contiguous per partition -> 8KB DMA descriptors)
            n_slabs = ROWS // 512  # 16; slabs [4m, 4m+4) belong to modality m
            xv = x[:].rearrange("(i p k) d -> i p k d", p=128, k=4)

            def _do_slab(i, xt):
                m, r = divmod(i, 4)
                s01 = wq.tile([128, D_LOC], f32, tag="s01", name="s01")
                s23 = wq.tile([128, D_LOC], f32, tag="s23", name="s23")
                nc.vector.tensor_add(s01[:], xt[:, 0, :], xt[:, 1, :])
                nc.vector.tensor_add(s23[:], xt[:, 2, :], xt[:, 3, :])
                ps = psc.tile([32, D_LOC], f32, tag="cps", name="cps")
                nc.tensor.matmul(ps[:], eq_sb[:], s01[:], start=True, stop=False)
                nc.tensor.matmul(ps[:], eq_sb[:], s23[:], start=False, stop=True)
                nc.scalar.copy(cen[m][r * 32 : (r + 1) * 32, :], ps[:])
                if r != 3:
                    return

                # modality m complete: transpose centers, Gram, sq-norms
                st = stats_tile(m)
                gcol = (m % 2) * 128 if m < 2 else 0
                for c in range(4):
                    pt = pst.tile([128, 128], f32, tag="pt", name="pt")
                    nc.tensor.transpose(
                        pt[:], cen[m][:, c * 128 : (c + 1) * 128], id_sb[:]
                    )
                    nc.scalar.copy(cT[m][:, c * 128 : (c + 1) * 128], pt[:])
                pg = psg.tile([128, 128], f32, tag="pg", name="pg")
                for c in range(4):
                    ct_chunk = cT[m][:, c * 128 : (c + 1) * 128]
                    nc.tensor.matmul(
                        pg[:], ct_chunk, ct_chunk, start=(c == 0), stop=(c == 3)
                    )
                sq = wp.tile([128, D_LOC], f32, tag="sq", name="sq")
                nc.scalar.activation(
                    sq[:],
                    cen[m][:],
                    Act.Square,
                    accum_out=st[:, scol(m) : scol(m) + 1],
                )
                # store H_part = s_part - G_part (linear in the partials, so
                # the AllReduce yields H = s_i - G directly; d2 = H + H^T)
                nc.scalar.activation(
                    st[:, gcol : gcol + 128],
                    pg[:],
                    Act.Identity,
                    bias=st[:, scol(m) : scol(m) + 1],
                    scale=-1.0,
                )

                if m == 1:
                    # modalities 0+1 done: reduction + post hide under sweep
                    if do_ar:
                        _all_reduce(stats_a, rst_a, W_A, "a")
                    _post_one(0)
                    _post_one(1)
                elif m == 2:
                    _pair_dp(0, 1, 2)
                    _pair_dp(1, 0, 2)
                    if do_ar:
                        _all_reduce(stats_b, rst_b, W_B, "b")
                    _post_one(2)
                elif m == 3:
                    _pair_dp(2, 0, 3)
                    _pair_dp(3, 1, 3)
                    if do_ar:
                        _all_gather_sum(stats_c, rst_c, W_B, "c")
                    _post_one(3)

            if parts == "dma":
                for mm in range(MODS):
                    nc.vector.memset(cen[mm][:], 0.0)
                nc.vector.memset(stats_a[:], 0.0)
                nc.vector.memset(stats_b[:], 0.0)
                nc.vector.memset(stats_c[:], 0.0)
                nc.vector.memset(anm[:], 1.0)
            pre_xts = None
            if parts == "compute":
                pre_xts = []
                for w in range(3):
                    pre_xt = xp.tile([128, 4, D_LOC], f32, tag="xt", name="xt")
                    nc.sync.dma_start(pre_xt[:], xv[w])
                    pre_xts.append(pre_xt)

            loop_cm = (
                tc.For_i(0, bench_reps, 1, staggered_reset=staggered)
                if bench_reps
                else contextlib.nullcontext()
            )
            loop_body = contextlib.ExitStack()
            loop_body.enter_context(loop_cm)

            for i in range(n_slabs):
                if parts == "compute":
                    xt = pre_xts[i % 3]
                else:
                    xt = xp.tile([128, 4, D_LOC], f32, tag="xt", name="xt")
                    if i < 2:
                        # fine-grained pieces for the first slabs so the
                        # add/matmul pipeline spins up before the full 1MB
                        # transfers complete
                        for k in range(4):
                            nc.sync.dma_start(xt[:, k, :], xv[i][:, k, :])
                    elif i >= n_slabs - 2:
                        # half-slab pieces at the end: the s01 add starts
                        # while the second half is still streaming, pulling
                        # the tail chain ~1us earlier
                        nc.sync.dma_start(xt[:, 0:2, :], xv[i][:, 0:2, :])
                        nc.sync.dma_start(xt[:, 2:4, :], xv[i][:, 2:4, :])
                    elif half_dma:
                        # each half feeds exactly one of the s01/s23 adds
                        nc.sync.dma_start(xt[:, 0:2, :], xv[i][:, 0:2, :])
                        nc.sync.dma_start(xt[:, 2:4, :], xv[i][:, 2:4, :])
                    else:
                        nc.sync.dma_start(xt[:], xv[i])
                if parts == "dma":
                    if i == 0:
                        _do_slab(0, xt)
                    continue
                _do_slab(i, xt)

            if parts != "dma":
                # --- diagonal (same-identity, cross-modality) distances ---
                for j, (a, b) in enumerate(PAIRS):
                    nc.vector.tensor_scalar(
                        pd[:, j : j + 1], dp_ap(j), -2.0, s_ap(a), Alu.mult, Alu.add
                    )
                    nc.vector.tensor_tensor(
                        pd[:, j : j + 1], pd[:, j : j + 1], s_ap(b), op=Alu.add
                    )
                nc.vector.tensor_scalar(pd[:], pd[:], 1.0e-12, None, Alu.max)
                nc.scalar.activation(pd[:], pd[:], Act.Sqrt)

                # --- margin-ranking relu terms, packed as 6 columns ---
                # (ap column in pd, an column, an source)
                terms = (
                    (0, 1, "pd"),   # mrl(an123, ap123)
                    (0, 2, "anm"),  # mrl(an33,  ap123)
                    (0, 0, "anm"),  # mrl(an11,  ap123)
                    (2, 3, "pd"),   # mrl(an124, ap124)
                    (2, 3, "anm"),  # mrl(an44,  ap124)
                    (2, 1, "anm"),  # mrl(an22,  ap124)
                )
                R = cenp.tile([128, 6], f32, tag="R", name="R")
                for jr, (apc, anc, src) in enumerate(terms):
                    an_col = pd if src == "pd" else anm
                    nc.vector.tensor_scalar(
                        R[:, jr : jr + 1], pd[:, apc : apc + 1],
                        an_col[:, anc : anc + 1], MARGIN,
                        Alu.subtract, Alu.add,
                    )
                nc.vector.tensor_scalar(R[:], R[:], 0.0, None, Alu.max)

                # --- means across the 128 identities + weighted combine ---
                pm = pss.tile([1, 6], f32, tag="pm", name="pm")
                nc.tensor.matmul(pm[:], on_sb[:], R[:], start=True, stop=True)
                fin = cenp.tile([1, 6], f32, tag="fin", name="fin")
                nc.vector.tensor_tensor(fin[:], pm[:], wv_sb[:], op=Alu.mult)
                lsb = cenp.tile([1, 1], f32, tag="lsb", name="lsb")
                nc.vector.tensor_reduce(
                    lsb[:], fin[:], axis=mybir.AxisListType.X, op=Alu.add
                )

            loop_body.close()

            if parts == "dma":
                nc.sync.dma_start(loss[:], cen[0][0:1, 0:1])
            else:
                nc.sync.dma_start(loss[:], lsb[:])

    nc.compile()
    return nc


def _get_program():
    global _PROGRAM
    if _PROGRAM is None:
        _PROGRAM = _build_program()
    return _PROGRAM


def kernel(inputs, targets=None, num_classes=None):
    from concourse import bass_utils

    x = np.ascontiguousarray(np.asarray(inputs, dtype=np.float32))
    assert x.shape == (ROWS, D_FULL), x.shape

    nc = _get_program()
    in_maps = [
        {"x0": np.ascontiguousarray(x[:, c * D_LOC : (c + 1) * D_LOC])}
        for c in range(N_CORES)
    ]
    res = bass_utils.run_bass_kernel_spmd(nc, in_maps, core_ids=list(range(N_CORES)))
    out = res.results[0]["loss"]
    return np.asarray(out, dtype=np.float32).reshape(())



# revision 45
# speedup vs baseline: 48436.5609x; 48436.5609x over previous
"""CPMLoss (cross-modal center / margin-ranking loss) on 8 Trainium2 NeuronCores.

Strategy (feature-dim sharding, fp8 transport):
  - The [8192, 4096] f32 input is cast to fp8e4m3 on the host (randn inputs:
    |x| < 6 is far inside e4m3 range; the quantization moves the final loss
    by ~2e-3 relative, versus a 2e-2 gate) and sharded along the feature dim
    D: core c gets the column slice [:, c*512:(c+1)*512] as 4.2 MB of fp8.
    This cuts both the host->device transfer and the per-core HBM sweep 4x
    versus f32.
  - Each core streams its slice once (4 modality-slabs of [128 partitions x
    16 rows x 512 cols], 4-8 KB contiguous DMA descriptors per partition)
    and computes, over its D-slice:
      * per-modality/identity centers as fp8 DoubleRow PE matmuls (0.5
        cycles/moving-row) accumulating in f32 PSUM via an indicator
        stationary eq[p, :, p] = 1/16; each modality's post-compute is
        emitted after the NEXT modality's matmuls so the PE never
        head-of-line blocks on the ACT PSUM->SBUF copy.
      * partial Gram matrices G_m = c_m @ c_m^T in bf16 (PE transposes into
        one PSUM tile, drained by a single DVE copy; matmul 1 cycle/row).
      * partial squared norms s_m (ACT Square + accumulate, f32).
      * partial cross-modality diagonal products dp_ab (bf16 DVE mult +
        reduce, 2x mode).
    All stats are linear in the D-partials and packed in ONE [128, 520]
    tile (4x128 H, 4 s, 4 dp).  Back-to-back collectives do not pipeline on
    this stack (2 stacked AllGathers cost 3-6x one), so a SINGLE bf16
    AllGather + 8-rank local sum at the end of the sweep performs the whole
    cross-core reduction; only it plus ~3 us of post-processing is exposed.
  - Host side: one cached jax-cpu jit does cast+reshuffle (~70 ms), and one
    cached jit(shard_map) runner executes the NEFF without re-tracing, so a
    warm kernel() call is dominated by shipping 33.5 MB over the axon
    tunnel.
  - Every core computes the same scalar; host takes core 0's.
"""

import numpy as np

for _p in ("/opt/trn_rl_repo",):
    import sys

    if _p not in sys.path:
        sys.path.append(_p)

ROWS = 8192          # 4 modalities x 128 identities x 16 samples
D_FULL = 4096
N_CORES = 8
D_LOC = D_FULL // N_CORES   # 512
P_ID = 128           # identities per modality
MODS = 4
K_SAMP = 16
MARGIN = 0.2
# (a, b) modality pairs whose diagonal distances feed the loss:
# j=0: d(c2,c3)=ap123, j=1: d(c1,c3)=an123, j=2: d(c1,c4)=ap124, j=3: d(c2,c4)=an124
PAIRS = ((1, 2), (0, 2), (0, 3), (1, 3))

_PROGRAM = None
_RUNNER = None
_CAST = None


def _build_program(bench_reps=0, xp_bufs=8, psc_bufs=2, parts="full", no_ar=False,
                   post="bf16", fuse_dp=False, eq_dtype="f8",
                   rpp=16, center="dr", dma_split=2, dma_q="sp", unroll=1,
                   chain=True, force_ar=False):
    import contextlib

    import ml_dtypes

    import concourse.mybir as mybir
    from concourse import bacc, tile

    f32 = mybir.dt.float32
    bf16 = mybir.dt.bfloat16
    f8 = mybir.dt.float8e4
    Alu = mybir.AluOpType
    Act = mybir.ActivationFunctionType

    nc = bacc.Bacc(
        "TRN2", target_bir_lowering=False, debug=False, num_devices=N_CORES
    )

    x = nc.dram_tensor("x0", [ROWS, D_LOC], f8, kind="ExternalInput")
    loss = nc.dram_tensor("loss", [1, 1], f32, kind="ExternalOutput")

    # --- constants baked into the NEFF ---
    # eq[p, r, r*32 + p//4] = 1/16: slab r of a modality holds rows
    # 512r + 4p .. 4p+4, i.e. partition p belongs to identity r*32 + p//4;
    # 16 accumulating fp8 matmuls (4 slabs x 4 row-slices) build the full
    # [128, 512] center block in one PSUM bank.  1/16 is exact in e4m3.
    eq_np_dt = (
        ml_dtypes.float8_e4m3 if eq_dtype == "f8" else ml_dtypes.bfloat16
    )
    # slab r of a modality holds rows r*128*rpp + rpp*p + k (k < rpp), all of
    # identity r*8*rpp + (rpp*p)//16; eq maps partition p to that identity.
    # For center="dr" the stationary operand carries the k-pair dim (both
    # halves identical) so one DoubleRow matmul consumes two row-slices.
    spm = ROWS // (4 * 128 * rpp)   # slabs per modality
    kdim = 2 if center == "dr" else 1
    eq_np = np.zeros((128, spm, kdim, 128), eq_np_dt)
    for p in range(128):
        for r in range(spm):
            eq_np[p, r, :, r * 8 * rpp + (rpp * p) // 16] = 1.0 / K_SAMP
    idb_np = np.eye(128, dtype=ml_dtypes.bfloat16)
    id_np = np.eye(128, dtype=np.float32)
    dg_np = np.zeros((128, 128), np.float32)
    np.fill_diagonal(dg_np, 1.0e30)
    on_np = np.ones((128, 1), np.float32)
    wv_np = (
        np.array([[0.5, 0.25, 0.25, 0.5, 0.25, 0.25]], np.float32) / 128.0
    )
    eq_d = nc.inline_tensor(eq_np, "eq_const")
    idb_d = nc.inline_tensor(idb_np, "idb_const")
    id_d = nc.inline_tensor(id_np, "id_const")
    dg_d = nc.inline_tensor(dg_np, "dg_const")
    on_d = nc.inline_tensor(on_np, "on_const")
    wv_d = nc.inline_tensor(wv_np, "wv_const")

    # single stats tile, reduced by ONE end-of-sweep AllGather+local-sum
    # (back-to-back collectives do not pipeline on this stack -- one big CC
    # op beats three staggered ones):
    #   [0:512) H0..H3 (128 cols each), [512:516) s0..s3, [516:520) dp0..dp3
    W_S = 520

    with tile.TileContext(nc) as tc:
        with (
            tc.tile_pool(name="constp", bufs=1) as constp,
            tc.tile_pool(name="cenp", bufs=1) as cenp,
            tc.tile_pool(name="xp", bufs=xp_bufs) as xp,
            tc.tile_pool(name="wp", bufs=2) as wp,
            tc.tile_pool(name="psc", bufs=psc_bufs, space="PSUM") as psc,
            tc.tile_pool(name="pst", bufs=2, space="PSUM") as pst,
            tc.tile_pool(name="psg", bufs=2, space="PSUM") as psg,
            tc.tile_pool(name="pss", bufs=1, space="PSUM") as pss,
            tc.tile_pool(name="dramp", bufs=1, space="DRAM") as dramp,
        ):
            cdt = bf16 if post == "bf16" else f32
            eq_sb = constp.tile(
                [128, spm, kdim, 128], f8 if eq_dtype == "f8" else bf16,
                tag="eq",
            )
            idb_sb = constp.tile([128, 128], bf16, tag="idb")
            id_sb = constp.tile([128, 128], f32, tag="id")
            dg_sb = constp.tile([128, 128], f32, tag="dg")
            on_sb = constp.tile([128, 1], f32, tag="on")
            wv_sb = constp.tile([1, 6], f32, tag="wv")
            nc.gpsimd.dma_start(eq_sb[:], eq_d[:])
            nc.gpsimd.dma_start(idb_sb[:], idb_d[:])
            nc.gpsimd.dma_start(id_sb[:], id_d[:])
            nc.gpsimd.dma_start(dg_sb[:], dg_d[:])
            nc.gpsimd.dma_start(on_sb[:], on_d[:])
            nc.gpsimd.dma_start(wv_sb[:], wv_d[:])

            do_ar = (
                (bench_reps == 0 or force_ar)
                and parts == "full"
                and not no_ar
            )

            def alloc_rep():
                T = {}
                T["cenh"] = [
                    cenp.tile([128, D_LOC], cdt, tag=f"cen{m}", name=f"cen{m}")
                    for m in range(MODS)
                ]
                T["cT"] = [
                    cenp.tile([128, D_LOC], cdt, tag=f"ct{m}", name=f"ct{m}")
                    for m in range(MODS)
                ]
                T["stats"] = cenp.tile([128, W_S], f32, tag="stats",
                                       name="stats")
                T["stats_h"] = cenp.tile([128, W_S], bf16, tag="stats_h",
                                         name="stats_h")
                T["rst"] = cenp.tile([128, W_S], f32, tag="rst", name="rst")
                T["rcat"] = cenp.tile([128, N_CORES, W_S], bf16, tag="rcat",
                                      name="rcat")
                T["anm"] = cenp.tile([128, 4], f32, tag="anm", name="anm")
                T["pd"] = cenp.tile([128, 4], f32, tag="pd", name="pd")
                T["ps_of"] = {}
                # bench mode (no_ar): collectives cannot live inside a For_i
                # loop; post-process local partials (same op costs).
                T["red"] = T["rst"] if do_ar else T["stats"]
                return T

            def scol(m):
                return 512 + m

            def g_ap(T, m):
                return T["red"][:, m * 128 : (m + 1) * 128]

            def s_ap(T, m):
                return T["red"][:, 512 + m : 513 + m]

            def dp_ap(T, j):
                return T["red"][:, 516 + j : 517 + j]

            def dp_store(T, j):
                return T["stats"], 516 + j

            def _all_gather_sum(T, sb_tile, dst, width, name):
                # transport in bf16 (one rounding of the H partials, ~0.04%
                # on d^2); the rank sum below runs in f32
                nc.vector.tensor_copy(T["stats_h"][:], sb_tile[:])
                ag_in = dramp.tile([128, width], bf16, tag=f"gi_{name}",
                                   name=f"gi_{name}")
                ag_out = dramp.tile([128 * N_CORES, width], bf16,
                                    tag=f"go_{name}", name=f"go_{name}")
                nc.gpsimd.dma_start(ag_in[:], T["stats_h"][:])
                nc.gpsimd.collective_compute(
                    "AllGather",
                    Alu.bypass,
                    replica_groups=[list(range(N_CORES))],
                    ins=[ag_in.opt()],
                    outs=[ag_out.opt()],
                )
                # ranks land on the partition axis [r*128+p, c]; bring them
                # side-by-side in the free dim and sum on DVE
                rcat = T["rcat"]
                nc.gpsimd.dma_start(
                    rcat[:], ag_out[:].rearrange("(r p) c -> p r c", r=N_CORES)
                )
                nc.vector.tensor_add(dst[:], rcat[:, 0, :], rcat[:, 1, :])
                for rr in range(2, N_CORES):
                    nc.vector.tensor_add(dst[:], dst[:], rcat[:, rr, :])

            def _all_reduce(T, sb_tile, dst, width, name):
                ar_in = dramp.tile([128, width], f32, tag=f"ai_{name}",
                                   name=f"ai_{name}")
                ar_out = dramp.tile([128, width], f32, tag=f"ao_{name}",
                                    name=f"ao_{name}")
                nc.gpsimd.dma_start(ar_in[:], sb_tile[:])
                nc.gpsimd.collective_compute(
                    "AllReduce",
                    Alu.add,
                    replica_groups=[list(range(N_CORES))],
                    ins=[ar_in.opt()],
                    outs=[ar_out.opt()],
                )
                nc.gpsimd.dma_start(dst[:], ar_out[:])

            def _post_one(T, m):
                # an_mm[m]; g_ap() holds H = s_i - G after AR; d2 = H + H^T.
                # min and sqrt commute (both monotone), so take the off-diag
                # row-min on d2 and sqrt only the [128,1] result.
                d = wp.tile([128, 128], f32, tag="d", name="d")
                pt = pst.tile([128, 128], f32, tag="pt", name="pt")
                anm = T["anm"]
                nc.tensor.transpose(pt[:], g_ap(T, m), id_sb[:])
                nc.vector.tensor_tensor(d[:], g_ap(T, m), pt[:], op=Alu.add)
                nc.vector.tensor_scalar(d[:], d[:], 1.0e-12, None, Alu.max)
                nc.vector.tensor_tensor(d[:], d[:], dg_sb[:], op=Alu.add)
                nc.vector.tensor_reduce(
                    anm[:, m : m + 1], d[:], axis=mybir.AxisListType.X, op=Alu.min
                )
                nc.scalar.activation(
                    anm[:, m : m + 1], anm[:, m : m + 1], Act.Sqrt
                )

            def _pair_dp(T, j, a, b):
                # (cenh[a] * cenh[b]) -> row-sum on DVE; all-bf16 operands
                # keep the DVE in its 2x mode ([128,1] f32 accumulator is
                # exempt as a scalar operand)
                pr = wp.tile([128, D_LOC], cdt, tag="pr", name="pr")
                st, col = dp_store(T, j)
                nc.vector.tensor_tensor(
                    pr[:], T["cenh"][a][:], T["cenh"][b][:], op=Alu.mult
                )
                nc.vector.tensor_reduce(
                    st[:, col : col + 1], pr[:],
                    axis=mybir.AxisListType.X, op=Alu.add,
                )

            # slab i (128*rpp rows), partition p holds rows
            # i*128*rpp + rpp*p .. +rpp (fully contiguous per partition ->
            # rpp*512B DMA descriptors)
            n_slabs = ROWS // (128 * rpp)
            xv = x[:].rearrange("(i p k) d -> i p k d", p=128, k=rpp)

            cur_ps = [None]

            def _do_slab(T, i, xt, prev_anm=None):
                m, r = divmod(i, spm)
                if r == 0:
                    cur_ps[0] = psc.tile([128, D_LOC], f32, tag="cps", name="cps")
                ps = cur_ps[0]
                if prev_anm is not None and i == 0:
                    # unrolled-latency bench: serialize reps by gating this
                    # rep's first PSUM write on the previous rep's post chain
                    nc.vector.tensor_scalar(
                        ps[:, 0:1], prev_anm[:, 3:4], 1.0e-30, None, Alu.mult
                    )
                if center == "dr":
                    for j in range(rpp // 2):
                        nc.tensor.matmul(
                            ps[:], eq_sb[:, r], xt[:, 2 * j : 2 * j + 2, :],
                            start=(r == 0 and j == 0),
                            stop=(r == spm - 1 and j == rpp // 2 - 1),
                            perf_mode=mybir.MatmulPerfMode.DoubleRow,
                        )
                else:
                    for k in range(rpp):
                        nc.tensor.matmul(
                            ps[:], eq_sb[:, r, 0], xt[:, k, :],
                            start=(r == 0 and k == 0),
                            stop=(r == spm - 1 and k == rpp - 1),
                        )
                if r != spm - 1:
                    return
                T["ps_of"][m] = ps

            def _mod_post(T, m):
                # modality m complete: centers to bf16 SBUF, transpose, Gram,
                # sq-norms.  Emitted AFTER modality m+1's center matmuls so
                # the PE never head-of-line blocks on the ACT copy.  ACT is
                # the scarce engine: it keeps only the PSUM->SBUF center copy
                # and Square(+accum); the batched transpose copy and the H
                # combine go to DVE.
                ps = T["ps_of"][m]
                cenh, cT = T["cenh"], T["cT"]
                nc.scalar.copy(cenh[m][:], ps[:])
                st = T["stats"]
                gcol = m * 128
                if cdt == bf16:
                    # all 4 chunk transposes fit one PSUM tile ([128,512]
                    # bf16 = 1KB/partition): one DVE copy drains them all
                    pt = pst.tile([128, D_LOC], cdt, tag="pt", name="pt")
                    for c in range(4):
                        nc.tensor.transpose(
                            pt[:, c * 128 : (c + 1) * 128],
                            cenh[m][:, c * 128 : (c + 1) * 128],
                            idb_sb[:],
                        )
                    nc.vector.tensor_copy(cT[m][:], pt[:])
                else:
                    for c in range(4):
                        pt = pst.tile([128, 128], cdt, tag="pt", name="pt")
                        nc.tensor.transpose(
                            pt[:], cenh[m][:, c * 128 : (c + 1) * 128],
                            id_sb[:],
                        )
                        nc.scalar.copy(cT[m][:, c * 128 : (c + 1) * 128], pt[:])
                pg = psg.tile([128, 128], f32, tag="pg", name="pg")
                for c in range(4):
                    ct_chunk = cT[m][:, c * 128 : (c + 1) * 128]
                    nc.tensor.matmul(
                        pg[:], ct_chunk, ct_chunk, start=(c == 0), stop=(c == 3)
                    )
                sq = wp.tile([128, D_LOC], f32, tag="sq", name="sq")
                nc.scalar.activation(
                    sq[:],
                    cenh[m][:],
                    Act.Square,
                    accum_out=st[:, scol(m) : scol(m) + 1],
                )
                # store H_part = s_part - G_part (linear in the partials, so
                # the AllReduce yields H = s_i - G directly; d2 = H + H^T)
                nc.vector.tensor_scalar(
                    st[:, gcol : gcol + 128], pg[:], -1.0,
                    st[:, scol(m) : scol(m) + 1], Alu.mult, Alu.add,
                )

                if m == 2:
                    # dp pairs needing only c1..c3 hide under m3's window
                    _pair_dp(T, 0, 1, 2)
                    _pair_dp(T, 1, 0, 2)
                elif m == 3:
                    _pair_dp(T, 2, 0, 3)
                    _pair_dp(T, 3, 1, 3)
                    if do_ar:
                        _all_gather_sum(T, T["stats"], T["rst"], W_S,
                                        f"s{T['rep']}")
                    for mm in range(MODS):
                        _post_one(T, mm)

            def _finish_slab(T, i):
                m, r = divmod(i, spm)
                if r == spm - 1:
                    _mod_post(T, m)

            def emit_rep(T, prev_anm=None):
                pd, anm = T["pd"], T["anm"]
                if parts == "dma":
                    for mm in range(MODS):
                        nc.vector.memset(T["cenh"][mm][:], 0.0)
                    nc.vector.memset(T["stats"][:], 0.0)
                    nc.vector.memset(anm[:], 1.0)
                pre_xts = None
                if parts == "compute":
                    pre_xts = []
                    for w in range(min(3, n_slabs)):
                        pre_xt = xp.tile([128, rpp, D_LOC], f8, tag="xt",
                                         name="xt")
                        nc.sync.dma_start(pre_xt[:], xv[w])
                        pre_xts.append(pre_xt)

                for i in range(n_slabs):
                    if parts == "compute":
                        xt = pre_xts[i % len(pre_xts)]
                    else:
                        xt = xp.tile([128, rpp, D_LOC], f8, tag="xt",
                                     name="xt")
                        # chunked transfers so compute pipelines within a
                        # slab; finer pieces for the first/last slab
                        # (pipeline spin-up / earlier tail start)
                        splits = (
                            4 if (i < 1 or i >= n_slabs - 1) else dma_split
                        )
                        splits = min(splits, rpp // 2)
                        step = rpp // splits
                        qs = (
                            [nc.sync, nc.gpsimd] if dma_q == "mix"
                            else [nc.sync]
                        )
                        if splits == 1:
                            qs[i % len(qs)].dma_start(xt[:], xv[i])
                        else:
                            for ai, a in enumerate(range(0, rpp, step)):
                                qs[(i * splits + ai) % len(qs)].dma_start(
                                    xt[:, a : a + step, :],
                                    xv[i][:, a : a + step, :],
                                )
                    if parts == "dma":
                        if i == 0:
                            _do_slab(T, 0, xt, prev_anm)
                        continue
                    _do_slab(T, i, xt, prev_anm)
                    if i > 0:
                        _finish_slab(T, i - 1)

                if parts == "dma":
                    return T["stats"][0:1, 0:1], anm
                _finish_slab(T, n_slabs - 1)

                # --- diagonal (same-identity, cross-modality) distances ---
                for j, (a, b) in enumerate(PAIRS):
                    nc.vector.tensor_scalar(
                        pd[:, j : j + 1], dp_ap(T, j), -2.0, s_ap(T, a),
                        Alu.mult, Alu.add,
                    )
                    nc.vector.tensor_tensor(
                        pd[:, j : j + 1], pd[:, j : j + 1], s_ap(T, b),
                        op=Alu.add,
                    )
                nc.vector.tensor_scalar(pd[:], pd[:], 1.0e-12, None, Alu.max)
                nc.scalar.activation(pd[:], pd[:], Act.Sqrt)

                # --- margin-ranking relu terms, packed as 6 columns ---
                # (ap column in pd, an column, an source)
                terms = (
                    (0, 1, "pd"),   # mrl(an123, ap123)
                    (0, 2, "anm"),  # mrl(an33,  ap123)
                    (0, 0, "anm"),  # mrl(an11,  ap123)
                    (2, 3, "pd"),   # mrl(an124, ap124)
                    (2, 3, "anm"),  # mrl(an44,  ap124)
                    (2, 1, "anm"),  # mrl(an22,  ap124)
                )
                R = cenp.tile([128, 6], f32, tag="R", name="R")
                for jr, (apc, anc, src) in enumerate(terms):
                    an_col = pd if src == "pd" else anm
                    nc.vector.tensor_scalar(
                        R[:, jr : jr + 1], pd[:, apc : apc + 1],
                        an_col[:, anc : anc + 1], MARGIN,
                        Alu.subtract, Alu.add,
                    )
                nc.vector.tensor_scalar(R[:], R[:], 0.0, None, Alu.max)

                # --- means across the 128 identities + weighted combine ---
                pm = pss.tile([1, 6], f32, tag="pm", name="pm")
                nc.tensor.matmul(pm[:], on_sb[:], R[:], start=True, stop=True)
                fin = cenp.tile([1, 6], f32, tag="fin", name="fin")
                nc.vector.tensor_tensor(fin[:], pm[:], wv_sb[:], op=Alu.mult)
                lsb = cenp.tile([1, 1], f32, tag="lsb", name="lsb")
                nc.vector.tensor_reduce(
                    lsb[:], fin[:], axis=mybir.AxisListType.X, op=Alu.add
                )
                return lsb[:], anm

            if bench_reps:
                T = alloc_rep()
                T["rep"] = 0
                with tc.For_i(0, bench_reps, 1):
                    out_ap, _ = emit_rep(T)
            else:
                prev_anm = None
                for rep in range(max(1, unroll)):
                    T = alloc_rep()
                    T["rep"] = rep
                    out_ap, prev_anm = emit_rep(
                        T, prev_anm if chain else None
                    )

            nc.sync.dma_start(loss[:], out_ap)

    nc.compile()
    return nc


def _get_program():
    global _PROGRAM
    if _PROGRAM is None:
        _PROGRAM = _build_program()
    return _PROGRAM


def _get_cast():
    """jax-cpu jit: f32 [8192,4096] -> fp8e4m3 [8*8192,512] in shard-concat
    layout (core c's D-slice at rows [c*8192, (c+1)*8192))."""
    global _CAST
    if _CAST is None:
        import jax
        import jax.numpy as jnp

        cpu = jax.devices("cpu")[0]
        _CAST = jax.jit(
            lambda a: a.astype(jnp.float8_e4m3)
            .reshape(ROWS, N_CORES, D_LOC)
            .transpose(1, 0, 2)
            .reshape(N_CORES * ROWS, D_LOC),
            device=cpu,
        )
    return _CAST


def make_runner(nc, n_cores):
    """Build a reusable jitted shard_map executor for a compiled Bass
    program (the per-call path of bass2jax.run_bass_via_pjrt, hoisted so the
    trace/compile happens once).  Returns (fn, mesh, in_names, out_names,
    zero_outs); fn takes the axis-0-concatenated per-core inputs followed by
    the (donated) zero-initialized output buffers."""
    import jax
    from jax.sharding import Mesh, PartitionSpec

    try:
        from jax import shard_map as _shard_map

        def shard_map(f, mesh, in_specs, out_specs, check_rep):
            return _shard_map(
                f, mesh=mesh, in_specs=in_specs, out_specs=out_specs,
                check_vma=check_rep,
            )
    except ImportError:
        from jax.experimental.shard_map import shard_map

    import concourse.mybir as mybir
    from concourse import bass2jax

    bass2jax.install_neuronx_cc_hook()

    partition_name = (
        nc.partition_id_tensor.name if nc.partition_id_tensor else None
    )
    in_names, out_names, out_avals, zero_outs = [], [], [], []
    for alloc in nc.m.functions[0].allocations:
        if not isinstance(alloc, mybir.MemoryLocationSet):
            continue
        name = alloc.memorylocations[0].name
        if alloc.kind == "ExternalInput":
            if name != partition_name:
                in_names.append(name)
        elif alloc.kind == "ExternalOutput":
            shape = tuple(alloc.tensor_shape)
            dtype = mybir.dt.np(alloc.dtype)
            out_names.append(name)
            out_avals.append(jax.core.ShapedArray(shape, dtype))
            zero_outs.append(np.zeros(shape, dtype))
    n_params, n_outs = len(in_names), len(out_avals)
    all_in_names = in_names + out_names + (
        [partition_name] if partition_name else []
    )

    def _body(*args):
        operands = list(args)
        if partition_name is not None:
            operands.append(bass2jax.partition_id_tensor())
        outs = bass2jax._bass_exec_p.bind(
            *operands,
            out_avals=tuple(out_avals),
            in_names=tuple(all_in_names),
            out_names=tuple(out_names),
            lowering_input_output_aliases=(),
            sim_require_finite=True,
            sim_require_nnan=True,
            nc=nc,
        )
        return tuple(outs)

    devices = jax.devices()[:n_cores]
    assert len(devices) == n_cores, (
        f"need {n_cores} devices, got {len(jax.devices())}"
    )
    mesh = Mesh(np.asarray(devices), ("core",))
    fn = jax.jit(
        shard_map(
            _body,
            mesh=mesh,
            in_specs=(PartitionSpec("core"),) * (n_params + n_outs),
            out_specs=(PartitionSpec("core"),) * n_outs,
            check_rep=False,
        ),
        donate_argnums=tuple(range(n_params, n_params + n_outs)),
        keep_unused=True,
    )
    return fn, mesh, in_names, out_names, zero_outs


def _get_runner():
    global _RUNNER
    if _RUNNER is None:
        nc = _get_program()
        fn, mesh, in_names, out_names, zero_outs = make_runner(nc, N_CORES)
        assert in_names == ["x0"] and out_names == ["loss"], (
            in_names, out_names,
        )
        concat_zeros = [
            np.zeros((N_CORES * z.shape[0], *z.shape[1:]), z.dtype)
            for z in zero_outs
        ]
        _RUNNER = (fn, concat_zeros)
    return _RUNNER


def kernel(inputs, targets=None, num_classes=None):
    x = np.asarray(inputs)
    assert x.shape == (ROWS, D_FULL), x.shape

    xq = np.asarray(_get_cast()(np.ascontiguousarray(x, dtype=np.float32)))
    try:
        fn, concat_zeros = _get_runner()
        out = fn(xq, *[z.copy() for z in concat_zeros])
        loss = np.asarray(out[0]).reshape(N_CORES, 1)[0, 0]
    except Exception:
        # fall back to the stock (per-call re-traced) executor
        from concourse import bass_utils

        nc = _get_program()
        in_maps = [
            {"x0": xq[c * ROWS : (c + 1) * ROWS]} for c in range(N_CORES)
        ]
        res = bass_utils.run_bass_kernel_spmd(
            nc, in_maps, core_ids=list(range(N_CORES))
        )
        loss = res.results[0]["loss"]
    return np.asarray(loss, dtype=np.float32).reshape(())
